# revision 8
# baseline (speedup 1.0000x reference)
"""Masked one-hot scatter kernel for Trainium2 (8 NeuronCores, SPMD).

Problem: out[b, n, c] = 1.0 if obj_labels[b, n] == c else 0.0
         (labels of -1 match no class -> all-zero row, which exactly
         reproduces the reference's valid-mask + one_hot(max(label, 0)))

The output depends only on obj_labels; obj_sem_cls_pred contributes shape
and dtype only, so it is never transferred to the device.

Strategy (data parallel over batch, no communication):
 - core i handles batches [32*i, 32*(i+1)) -> 4096 output rows of 607
 - rows are processed in G=8 groups of 512; within a group, partition p
   computes rows 512g + 4p + {0..3} via 4 DVE tensor_scalar(is_equal)
   ops against a constant iota row (free dim padded 607->608 so fp32
   tensor_scalar hits the DVE 2x perf mode); the per-row label is a
   per-partition scalar, host-pregathered into the matching layout
 - each group's tile [128, 4*608] DMAs out as ONE fully-contiguous
   1.24 MB HBM write on its own HWDGE lane (this walrus build allows a
   single sync-wait per instruction, so no DMA lane may be used twice)
 - ~9.9 MB of output writes per core is the roofline (~28 us @ 358 GB/s)
"""

import os

import numpy as np

from concourse import bacc, mybir
from concourse.bass_utils import run_bass_kernel_spmd
from concourse.tile import TileContext

B, N, C = 256, 128, 607
NCORES = 8
BS = B // NCORES  # batches per core
ROWS = BS * N  # 4096 output rows per core
CPAD = 608  # even free dim -> fp32 tensor_scalar 2x perf mode
G = 8  # output groups = one HWDGE DMA lane each
RPG = ROWS // G  # 512 rows per group
JPG = RPG // N  # 4 rows per partition per group

_graph_cache = None
last_result = None  # BassKernelResults of the most recent run (for test harness)


def _build_graph():
    # Bacc (not raw Bass): its compile()/finalize() pipeline runs
    # generate_event_semaphores(), which splits multi-sem waits into
    # event-semaphore instructions -- walrus rejects >1 wait per inst.
    nc = bacc.Bacc()
    # consts = [iota | labels]: a single input DMA keeps every
    # tensor_scalar at <=1 sync wait (walrus limit on this build)
    consts_p = nc.declare_dram_parameter(
        "consts", [N, CPAD + BS], mybir.dt.float32, isOutput=False
    )
    out_p = nc.declare_dram_parameter(
        "out", [ROWS, C], mybir.dt.float32, isOutput=True
    )

    f32 = mybir.dt.float32
    out_g = out_p[:].rearrange("(g p j) c -> g p j c", g=G, p=N)  # [G, 128, 4, 607]
    with TileContext(nc) as tc:
        with tc.tile_pool(name="const", bufs=1) as cpool:
            with tc.tile_pool(name="work", bufs=G) as wpool:
                consts_t = cpool.tile([N, CPAD + BS], f32)
                # SWDGE so the 8 HWDGE lanes stay single-use (outputs only)
                nc.gpsimd.dma_start(out=consts_t, in_=consts_p[:])
                iota_t = consts_t[:, 0:CPAD]
                lab_t = consts_t[:, CPAD : CPAD + BS]
                for g in range(G):
                    t = wpool.tile([N, JPG * CPAD], f32)
                    for j in range(JPG):
                        r = g * JPG + j
                        nc.vector.tensor_scalar(
                            t[:, j * CPAD : (j + 1) * CPAD],
                            iota_t,
                            lab_t[:, r : r + 1],
                            None,
                            mybir.AluOpType.is_equal,
                        )
                    src = t.rearrange("p (j c) -> p j c", c=CPAD)[:, :, 0:C]
                    nc.sync.dma_start(out=out_g[g], in_=src)
    nc.finalize()
    return nc


def kernel(obj_sem_cls_pred=None, obj_labels=None, cur_step=None, total_steps=None, **_):
    global _graph_cache, last_result
    labels = np.asarray(obj_labels)
    assert labels.shape == (B, N), labels.shape

    if _graph_cache is None:
        _graph_cache = _build_graph()
    nc = _graph_cache

    iota_arr = np.broadcast_to(np.arange(CPAD, dtype=np.float32), (N, CPAD))
    # labels_t[p, r] = label of output row 512*(r//4) + 4*p + (r%4)
    rr = np.arange(BS)
    pp = np.arange(N)[:, None]
    gather_idx = RPG * (rr // JPG)[None, :] + JPG * pp + (rr % JPG)[None, :]

    in_maps = []
    for c in range(NCORES):
        lf = labels[c * BS : (c + 1) * BS].reshape(-1)  # [4096]
        lab_t = lf[gather_idx].astype(np.float32)  # [N, BS]
        consts = np.ascontiguousarray(
            np.concatenate([iota_arr, lab_t], axis=1)
        )  # [N, CPAD + BS]
        in_maps.append({"consts": consts})

    trace = bool(os.environ.get("KERNEL_TRACE"))
    last_result = run_bass_kernel_spmd(
        nc, in_maps, core_ids=list(range(NCORES)), trace=trace
    )
    out = np.concatenate(
        [last_result.results[i]["out"].reshape(BS, N, C) for i in range(NCORES)],
        axis=0,
    )
    return np.ascontiguousarray(out)


# revision 15
# speedup vs baseline: 1.9469x; 1.9469x over previous
"""Masked one-hot scatter kernel for Trainium2 (8 NeuronCores, SPMD).

Problem: out[b, n, c] = 1.0 if obj_labels[b, n] == c else 0.0
         (labels of -1 match no class -> all-zero row, which exactly
         reproduces the reference's valid-mask + one_hot(max(label, 0)))

The output depends only on obj_labels; obj_sem_cls_pred contributes shape
and dtype only, so it is never transferred to the device.

Strategy (data parallel over batch, no communication). Per core: 4096
output rows of 607 floats (~9.9 MB of HBM writes = the roofline, ~28 us
at ~358 GB/s). The kernel is one DVE pass + 4 fully-contiguous stores:

 - rows are split into 4 chunks of [1,2,2,3]/8 of the output; within a
   chunk, partition p owns F consecutive rows (F = 4/8/8/12)
 - one DVE tensor_scalar(is_equal) per row-per-partition group compares
   a constant iota row against the per-partition label (host-pregathered
   into the matching [128, 32] layout)
 - overlap-write trick: each ts writes 608 elements (even free dim ->
   fp32 2x DVE perf mode) at stride 607, stomping the next row's first
   element with a 0; in-order DVE execution repairs it on the next op.
   This keeps the tile 607-packed so the DMA source is contiguous --
   a strided (608-padded) source halves DMA store bandwidth.
 - each chunk DMAs out as ONE contiguous store (src and dst both fully
   contiguous); measured steady-state: ~28 us/iter = ~354 GB/s/core.
   Small first chunk starts the DMA stream after ~1/8 of the compute.
"""

import os

import numpy as np

from concourse import bacc, mybir
from concourse.bass_utils import run_bass_kernel_spmd
from concourse.tile import TileContext

B, N, C = 256, 128, 607
NCORES = 8
BS = B // NCORES  # batches per core
ROWS = BS * N  # 4096 output rows per core
CPAD = 608  # even free dim -> fp32 tensor_scalar 2x perf mode
# DMA chunk schedule: rows-per-partition per chunk. Small chunks first so
# the DMA stream starts ~2.3us in; a big last chunk keeps store bandwidth
# near line rate (bigger DMAs -> fewer per-DMA overheads).
F_LIST = [4, 8, 8, 12]
BASES = [0, 512, 1536, 2560]  # first output row of each chunk (128*cumsum)

_graph_cache = None
last_result = None  # BassKernelResults of the most recent run (for test harness)


def _build_graph():
    # Bacc (not raw Bass): its finalize() pipeline runs
    # generate_event_semaphores(), which splits multi-sem waits into
    # event-semaphore instructions -- walrus rejects >1 wait per inst.
    nc = bacc.Bacc()
    # consts = [iota | labels]: a single input DMA keeps the first
    # tensor_scalar at one sync wait
    consts_p = nc.declare_dram_parameter(
        "consts", [N, CPAD + BS], mybir.dt.float32, isOutput=False
    )
    out_p = nc.declare_dram_parameter(
        "out", [ROWS, C], mybir.dt.float32, isOutput=True
    )

    f32 = mybir.dt.float32
    with TileContext(nc) as tc:
        with tc.tile_pool(name="const", bufs=1) as cpool:
            with tc.tile_pool(name="work", bufs=1) as wpool:
                consts_t = cpool.tile([N, CPAD + BS], f32)
                nc.sync.dma_start(out=consts_t, in_=consts_p[:])
                iota_t = consts_t[:, 0:CPAD]
                lab_t = consts_t[:, CPAD : CPAD + BS]
                col = 0
                for ci, F in enumerate(F_LIST):
                    nelem = F * C
                    t = wpool.tile(
                        [N, nelem + 1], f32, tag=f"t{ci}", name=f"t{ci}"
                    )
                    for f in range(F):
                        nc.vector.tensor_scalar(
                            t[:, f * C : f * C + CPAD],
                            iota_t,
                            lab_t[:, col : col + 1],
                            None,
                            mybir.AluOpType.is_equal,
                        )
                        col += 1
                    nrows = N * F
                    dst = out_p[BASES[ci] : BASES[ci] + nrows].rearrange(
                        "(p f) c -> p (f c)", p=N
                    )
                    # ACT's HWDGE ring; the consts load is on SP's
                    nc.scalar.dma_start(out=dst, in_=t[:, 0:nelem])
    nc.finalize()
    return nc


def _build_graph_raw():
    """Raw-Bacc variant: manual semaphores, no Tile scheduling layer.

    SP loads consts; DVE runs the 32 compares back-to-back (in-order, so
    the overlap-write trick needs no intra-engine sync), bumping a chunk
    semaphore after each chunk's last op; ACT waits per chunk and issues
    the 4 contiguous stores, then waits for all stores to land.
    """
    nc = bacc.Bacc()
    consts_p = nc.declare_dram_parameter(
        "consts", [N, CPAD + BS], mybir.dt.float32, isOutput=False
    )
    out_p = nc.declare_dram_parameter(
        "out", [ROWS, C], mybir.dt.float32, isOutput=True
    )
    f32 = mybir.dt.float32

    from contextlib import ExitStack

    with ExitStack() as stack:
        consts_t = stack.enter_context(
            nc.sbuf_tensor("consts_t", [N, CPAD + BS], f32)
        )
        tiles = [
            stack.enter_context(nc.sbuf_tensor(f"t{ci}", [N, F * C + 1], f32))
            for ci, F in enumerate(F_LIST)
        ]
        s_in = stack.enter_context(nc.semaphore("s_in"))
        s_chunk = stack.enter_context(nc.semaphore("s_chunk"))
        s_out = stack.enter_context(nc.semaphore("s_out"))
        block = stack.enter_context(nc.Block())
        iota_t = consts_t[:, 0:CPAD]
        lab_t = consts_t[:, CPAD : CPAD + BS]

        @block.sync
        def _(sync):
            sync.dma_start(out=consts_t[:], in_=consts_p[:]).then_inc(s_in, 16)

        @block.vector
        def _(vector):
            vector.wait_ge(s_in, 16)
            col = 0
            for ci, F in enumerate(F_LIST):
                t = tiles[ci]
                for f in range(F):
                    ins = nc.vector.tensor_scalar(
                        t[:, f * C : f * C + CPAD],
                        iota_t,
                        lab_t[:, col : col + 1],
                        None,
                        mybir.AluOpType.is_equal,
                    )
                    col += 1
                ins.then_inc(s_chunk, 1)

        @block.scalar
        def _(scalar):
            for ci, F in enumerate(F_LIST):
                scalar.wait_ge(s_chunk, ci + 1)
                nrows = N * F
                dst = out_p[BASES[ci] : BASES[ci] + nrows].rearrange(
                    "(p f) c -> p (f c)", p=N
                )
                scalar.dma_start(out=dst, in_=tiles[ci][:, 0 : F * C]).then_inc(
                    s_out, 16
                )
            scalar.wait_ge(s_out, 16 * len(F_LIST))

    nc.finalize()
    return nc


def kernel(obj_sem_cls_pred=None, obj_labels=None, cur_step=None, total_steps=None, **_):
    global _graph_cache, last_result
    labels = np.asarray(obj_labels)
    assert labels.shape == (B, N), labels.shape

    if _graph_cache is None:
        if os.environ.get("KERNEL_IMPL", "raw") == "raw":
            _graph_cache = _build_graph_raw()
        else:
            _graph_cache = _build_graph()
    nc = _graph_cache

    iota_arr = np.broadcast_to(np.arange(CPAD, dtype=np.float32), (N, CPAD))
    in_maps = []
    for c in range(NCORES):
        lf = labels[c * BS : (c + 1) * BS].reshape(-1)  # [4096] local flat rows
        lab_t = np.empty((N, BS), np.float32)
        col = 0
        for ci, F in enumerate(F_LIST):
            base = BASES[ci]
            lab_t[:, col : col + F] = lf[base : base + N * F].reshape(N, F)
            col += F
        consts = np.ascontiguousarray(
            np.concatenate([iota_arr, lab_t], axis=1)
        )  # [N, CPAD + BS]
        in_maps.append({"consts": consts})

    trace = bool(os.environ.get("KERNEL_TRACE"))
    last_result = run_bass_kernel_spmd(
        nc, in_maps, core_ids=list(range(NCORES)), trace=trace
    )
    out = np.concatenate(
        [last_result.results[i]["out"].reshape(BS, N, C) for i in range(NCORES)],
        axis=0,
    )
    return np.ascontiguousarray(out)


# revision 21
# speedup vs baseline: 1.9876x; 1.0209x over previous
"""Masked one-hot scatter kernel for Trainium2 (8 NeuronCores, SPMD).

Problem: out[b, n, c] = 1.0 if obj_labels[b, n] == c else 0.0
         (labels of -1 match no class -> all-zero row, which exactly
         reproduces the reference's valid-mask + one_hot(max(label, 0)))

The output depends only on obj_labels; obj_sem_cls_pred contributes shape
and dtype only, so it is never transferred to the device.

Strategy (data parallel over batch, no communication). Per core: 4096
output rows of 607 floats (~9.9 MB of HBM writes = the roofline, ~28 us
at ~358 GB/s). The kernel is one DVE pass + 4 fully-contiguous stores:

 - rows are split into 4 chunks of [1,2,2,3]/8 of the output; within a
   chunk, partition p owns F consecutive rows (F = 4/8/8/12)
 - one DVE tensor_scalar(is_equal) per row-per-partition group compares
   a constant iota row against the per-partition label (host-pregathered
   into the matching [128, 32] layout)
 - overlap-write trick: each ts writes 608 elements (even free dim ->
   fp32 2x DVE perf mode) at stride 607, stomping the next row's first
   element with a 0; in-order DVE execution repairs it on the next op.
   This keeps the tile 607-packed so the DMA source is contiguous --
   a strided (608-padded) source halves DMA store bandwidth.
 - each chunk DMAs out as ONE contiguous store (src and dst both fully
   contiguous); measured steady-state: ~28 us/iter = ~354 GB/s/core.
   Small first chunk starts the DMA stream after ~1/8 of the compute.
"""

import os

import numpy as np

from concourse import bacc, mybir
from concourse.bass_utils import run_bass_kernel_spmd
from concourse.tile import TileContext

B, N, C = 256, 128, 607
NCORES = 8
BS = B // NCORES  # batches per core
ROWS = BS * N  # 4096 output rows per core
CPAD = 608  # even free dim -> fp32 tensor_scalar 2x perf mode
# DMA chunk schedule: rows-per-partition per chunk. Small chunks first so
# the DMA stream starts ~2.3us in; a big last chunk keeps store bandwidth
# near line rate (bigger DMAs -> fewer per-DMA overheads).
F_LIST = [4, 8, 8, 12]
BASES = [0, 512, 1536, 2560]  # first output row of each chunk (128*cumsum)

_graph_cache = None
last_result = None  # BassKernelResults of the most recent run (for test harness)


def _build_graph():
    # Bacc (not raw Bass): its finalize() pipeline runs
    # generate_event_semaphores(), which splits multi-sem waits into
    # event-semaphore instructions -- walrus rejects >1 wait per inst.
    nc = bacc.Bacc()
    iota_p = nc.declare_dram_parameter(
        "iota16", [N, CPAD], mybir.dt.float16, isOutput=False
    )
    labels_p = nc.declare_dram_parameter(
        "labels", [N, BS], mybir.dt.float32, isOutput=False
    )
    out_p = nc.declare_dram_parameter(
        "out", [ROWS, C], mybir.dt.float32, isOutput=True
    )

    f32 = mybir.dt.float32
    with TileContext(nc) as tc:
        with tc.tile_pool(name="const", bufs=1) as cpool:
            with tc.tile_pool(name="work", bufs=1) as wpool:
                iota_t = cpool.tile([N, CPAD], mybir.dt.float16)
                nc.sync.dma_start(out=iota_t, in_=iota_p[:])
                lab_t = cpool.tile([N, BS], f32)
                nc.sync.dma_start(out=lab_t, in_=labels_p[:])
                col = 0
                for ci, F in enumerate(F_LIST):
                    nelem = F * C
                    t = wpool.tile(
                        [N, nelem + 1], f32, tag=f"t{ci}", name=f"t{ci}"
                    )
                    for f in range(F):
                        nc.vector.tensor_scalar(
                            t[:, f * C : f * C + CPAD],
                            iota_t,
                            lab_t[:, col : col + 1],
                            None,
                            mybir.AluOpType.is_equal,
                        )
                        col += 1
                    nrows = N * F
                    dst = out_p[BASES[ci] : BASES[ci] + nrows].rearrange(
                        "(p f) c -> p (f c)", p=N
                    )
                    # ACT's HWDGE ring; the consts load is on SP's
                    nc.scalar.dma_start(out=dst, in_=t[:, 0:nelem])
    nc.finalize()
    return nc


def _build_graph_raw():
    """Raw-Bacc variant: manual semaphores, no Tile scheduling layer.

    SP loads consts; DVE runs the 32 compares back-to-back (in-order, so
    the overlap-write trick needs no intra-engine sync), bumping a chunk
    semaphore after each chunk's last op; ACT waits per chunk and issues
    the 4 contiguous stores, then waits for all stores to land.
    """
    nc = bacc.Bacc()
    # fp16 iota (exact for integers <= 2048) halves the startup DMA; the
    # per-partition scalar operand must stay float32 (bass asserts it)
    iota_p = nc.declare_dram_parameter(
        "iota16", [N, CPAD], mybir.dt.float16, isOutput=False
    )
    labels_p = nc.declare_dram_parameter(
        "labels", [N, BS], mybir.dt.float32, isOutput=False
    )
    out_p = nc.declare_dram_parameter(
        "out", [ROWS, C], mybir.dt.float32, isOutput=True
    )
    f32 = mybir.dt.float32

    from contextlib import ExitStack

    with ExitStack() as stack:
        iota_t = stack.enter_context(
            nc.sbuf_tensor("iota_t", [N, CPAD], mybir.dt.float16)
        )
        lab_t = stack.enter_context(nc.sbuf_tensor("lab_t", [N, BS], f32))
        tiles = [
            stack.enter_context(nc.sbuf_tensor(f"t{ci}", [N, F * C + 1], f32))
            for ci, F in enumerate(F_LIST)
        ]
        s_in = stack.enter_context(nc.semaphore("s_in"))
        s_chunk = stack.enter_context(nc.semaphore("s_chunk"))
        s_out = stack.enter_context(nc.semaphore("s_out"))
        block = stack.enter_context(nc.Block(no_gpsimd_drain=True))

        @block.sync
        def _(sync):
            sync.dma_start(out=iota_t[:], in_=iota_p[:]).then_inc(s_in, 16)

        @block.vector
        def _(vector):
            vector.wait_ge(s_in, 32)
            col = 0
            for ci, F in enumerate(F_LIST):
                t = tiles[ci]
                for f in range(F):
                    ins = nc.vector.tensor_scalar(
                        t[:, f * C : f * C + CPAD],
                        iota_t[:],
                        lab_t[:, col : col + 1],
                        None,
                        mybir.AluOpType.is_equal,
                    )
                    col += 1
                ins.then_inc(s_chunk, 1)

        @block.scalar
        def _(scalar):
            # labels load on ACT's ring, in parallel with iota on SP's
            scalar.dma_start(out=lab_t[:], in_=labels_p[:]).then_inc(s_in, 16)
            for ci, F in enumerate(F_LIST):
                scalar.wait_ge(s_chunk, ci + 1)
                nrows = N * F
                dst = out_p[BASES[ci] : BASES[ci] + nrows].rearrange(
                    "(p f) c -> p (f c)", p=N
                )
                scalar.dma_start(out=dst, in_=tiles[ci][:, 0 : F * C]).then_inc(
                    s_out, 16
                )
            scalar.wait_ge(s_out, 16 * len(F_LIST))

    nc.finalize()
    return nc


def kernel(obj_sem_cls_pred=None, obj_labels=None, cur_step=None, total_steps=None, **_):
    global _graph_cache, last_result
    labels = np.asarray(obj_labels)
    assert labels.shape == (B, N), labels.shape

    if _graph_cache is None:
        if os.environ.get("KERNEL_IMPL", "raw") == "raw":
            _graph_cache = _build_graph_raw()
        else:
            _graph_cache = _build_graph()
    nc = _graph_cache

    iota16 = np.ascontiguousarray(
        np.broadcast_to(np.arange(CPAD, dtype=np.float16), (N, CPAD))
    )
    in_maps = []
    for c in range(NCORES):
        lf = labels[c * BS : (c + 1) * BS].reshape(-1)  # [4096] local flat rows
        lab_t = np.empty((N, BS), np.float32)
        col = 0
        for ci, F in enumerate(F_LIST):
            base = BASES[ci]
            lab_t[:, col : col + F] = lf[base : base + N * F].reshape(N, F)
            col += F
        in_maps.append({"iota16": iota16, "labels": lab_t})

    trace = bool(os.environ.get("KERNEL_TRACE"))
    last_result = run_bass_kernel_spmd(
        nc, in_maps, core_ids=list(range(NCORES)), trace=trace
    )
    out = np.concatenate(
        [last_result.results[i]["out"].reshape(BS, N, C) for i in range(NCORES)],
        axis=0,
    )
    return np.ascontiguousarray(out)
